# revision 44
# baseline (speedup 1.0000x reference)
"""Trainium2 Bass kernel for a post-LN transformer block (MHA + FFN).

Contract: kernel(**inputs) takes the FULL unsharded inputs (as produced by
the problem's setup_inputs) and returns the FULL output [2, 2048, 1024].

Sharding: token-parallel across 8 cores. Core c handles 512 tokens of
batch c//4. K^T and V are all-gathered per head-pair (8 small AllGathers
on separate DRAM tensors, launched as early as possible and overlapped
with Q projection + attention) within each 4-core replica group.

Attention AV uses [V|ones] 128-col stationaries so softmax denominators
accumulate in the same PSUM banks as the context (no partition
broadcast / gather needed for the normalize).

Matmuls run in bf16 (fp32 PSUM accumulation).
"""
import sys

for _p in ('/opt/trn_rl_repo', '/opt/pypackages'):
    if _p not in sys.path:
        sys.path.insert(0, _p)

import numpy as np
import ml_dtypes
import concourse.bass as bass
import concourse.tile as tile
from concourse import bacc, mybir
from concourse.bass import ts
from concourse.masks import make_identity
from contextlib import ExitStack

# ---- profiling shim (enables trace=True under axon; harmless if unused) ----
def _install_prof_shim():
    import types
    if 'antenv.axon_hooks' in sys.modules:
        return
    try:
        import trn_agent_boot.trn_boot as tb
        hook = tb._ntff_profile_via_ctypes('/opt/axon/libaxon_pjrt.so')
    except Exception:
        hook = None
    mod = types.ModuleType('antenv.axon_hooks')
    mod.get_axon_ntff_profile_hook = lambda: hook
    mod.set_axon_ntff_profile_hook = lambda h: None
    sys.modules['antenv.axon_hooks'] = mod

_install_prof_shim()

from concourse.bass_utils import run_bass_kernel_spmd  # noqa: E402

B, S, H, NH, HD = 2, 2048, 1024, 16, 64
P = 128
NCORES = 8
GSIZE = 4                    # replica-group size (cores per batch)
TQ = S // GSIZE              # tokens per core = 512
FT = H // P                  # feature tiles = 8
MT = TQ // P                 # token tiles per core = 4
EPS = 1e-5
RG = [[0, 1, 2, 3], [4, 5, 6, 7]]
VB = MT * HD                 # one head's V block per rank = 256 elems/partition
BW = TQ + 2 * VB             # bounce row bytes (all fp8): K | V0 | V1
NEX = 384                    # queries with exact (scalar-engine) exp
# Schraudolph fast-exp constants (bf16-truncated minimax, tuned offline)
EXP_A = float(np.float32(12102203.161561485))    # 2^23 / ln 2
EXP_B = float(np.float32(127 * 2 ** 23 - 479500 + 0.5))

f32 = mybir.dt.float32
bf16 = mybir.dt.bfloat16
i32 = mybir.dt.int32
fp8 = mybir.dt.float8e4
AF = mybir.ActivationFunctionType
ALU = mybir.AluOpType

DEBUG = False


def build_kernel():
    nc = bacc.Bacc("TRN2", target_bir_lowering=False, debug=False,
                   num_devices=NCORES)

    def din(name, shape, dt=f32):
        return nc.dram_tensor(name, shape, dt, kind="ExternalInput").ap()

    # inputs (per-core values supplied via in_maps)
    xT = din("xT", [H, TQ], bf16)           # x slice, transposed [feat, tok]
    xnb = din("xnb", [TQ, H])               # x slice natural + bo pre-added
    wqT = din("wqT", [H, H], bf16)          # Wq.T  [in, out]
    wkTs = din("wkTs", [H, H], bf16)        # Wk.T * 0.125
    wvT = din("wvT", [H, H], bf16)
    woT = din("woT", [H, H], bf16)
    w1T = din("w1T", [H, H], bf16)          # (W1 * g1).T  (gamma folded)
    w2T = din("w2T", [H, H], bf16)
    bqp = din("bqp", [P, FT])               # bq as [part, tile]
    bkp = din("bkp", [P, FT])               # bk * 0.125
    b1p = din("b1p", [P, FT])               # b1 + W1 @ be1
    bvB = din("bvB", [P, H])                # broadcast rows
    g1B = din("g1B", [P, H], bf16)
    bb2B = din("bb2B", [P, H], bf16)        # be1 + b2
    g2B = din("g2B", [P, H])
    be2B = din("be2B", [P, H])
    nri = din("nri", [P, 2], i32)           # [1, -1] int scalars
    nrm = din("nrm", [P, 1], i32)           # rsqrt magic + 1
    y = nc.dram_tensor("y", [TQ, H], f32, kind="ExternalOutput").ap()

    bounce = [nc.dram_tensor(f"bounce{t}", [P, BW], fp8).ap()
              for t in range(FT)]
    agout = [nc.dram_tensor(f"agout{t}", [GSIZE, P, BW], fp8).ap()
             for t in range(FT)]

    dbg = {}
    if DEBUG:
        for nm, shp, dt in [("dqt", [P, FT, TQ], bf16),
                            ("dqt2", [P, FT, TQ], bf16),
                            ("dctx", [P, FT, TQ], bf16),
                            ("dt1", [P, MT, H], f32),
                            ("dzn", [P, MT, H], bf16),
                            ("dht", [P, FT, TQ], bf16),
                            ("dag0", [GSIZE, P, BW], fp8),
                            ("dpc", [P, 2, TQ], f32),
                            ("drec", [P, TQ], f32)]:
            dbg[nm] = nc.dram_tensor(nm, shp, dt, kind="ExternalOutput").ap()

    with tile.TileContext(nc) as tc, ExitStack() as ctx:
        # ---------------- persistent pools ----------------
        const = ctx.enter_context(tc.tile_pool(name="const", bufs=1))
        acts = ctx.enter_context(tc.tile_pool(name="acts", bufs=1))
        wres = ctx.enter_context(tc.tile_pool(name="wres", bufs=1))
        wpool = ctx.enter_context(tc.tile_pool(name="w", bufs=3))
        vab = ctx.enter_context(tc.tile_pool(name="vab", bufs=1))

        # exp table warm-up on the scalar queue (ACT_TABLE_LOAD ~2.7us)
        eps_s = const.tile([P, 1], f32)
        nc.vector.memset(eps_s[:], EPS)
        warm_s = const.tile([P, 1], f32)
        nc.scalar.activation(warm_s[:], eps_s[:], AF.Exp)
        # identity for PE transposes (bf16 to match zn dtype)
        ident = const.tile([P, P], bf16)
        make_identity(nc, ident)

        bq_s = const.tile([P, FT], f32)
        bk_s = const.tile([P, FT], f32)
        b1_s = const.tile([P, FT], f32)
        bvB_s = const.tile([P, H], f32)
        nri_s = const.tile([P, 2], i32)
        nrm_s = const.tile([P, 1], i32)

        # deferred constants + resident weights (prefetched mid-attention)
        xnb_s = acts.tile([P, MT, H], f32)
        g1B_s = const.tile([P, H], bf16)
        bb2B_s = const.tile([P, H], bf16)
        g2B_s = const.tile([P, H], f32)
        be2B_s = const.tile([P, H], f32)
        wo_s = [wres.tile([P, FT, 512], bf16, name=f"wo{h}", tag=f"wo{h}")
                for h in range(2)]
        w2_s = [wres.tile([P, FT, 512], bf16, name=f"w2{h}", tag=f"w2{h}")
                for h in range(2)]

        # resident activations
        qt_s = acts.tile([P, FT, TQ], fp8)       # Q^T
        ctxT_s = acts.tile([P, FT, TQ], bf16)    # attention ctx^T (normalized)
        zn_s = acts.tile([P, MT, H], bf16)       # LN1 center-scaled (pre-gamma)
        ln1T_s = acts.tile([P, FT, TQ], bf16)    # zn transposed
        hT_s = acts.tile([P, FT, TQ], bf16)      # relu(fc1), transposed
        ln1b2_s = acts.tile([P, MT, H], bf16)    # zn*g1 + (be1+b2)  residual

        # V stationaries for AV, fp8 (double-buffered by rb parity), plus a
        # persistent ones tile: ctx and denominator halves of each psC bank
        # are computed by two col-tiled (concurrent) m=64 matmuls.
        vA_s = [vab.tile([P, MT, HD], fp8, name=f"vA{i}", tag=f"vA{i}")
                for i in range(2)]
        vB_s = [vab.tile([P, MT, HD], fp8, name=f"vB{i}", tag=f"vB{i}")
                for i in range(2)]
        ones64 = const.tile([P, HD], fp8)
        nc.vector.memset(ones64[:], 1.0)

        # ------------- phase A: K^T, V projections + 8 AllGathers ----------
        with tc.tile_pool(name="phA", bufs=1) as phA, \
             tc.tile_pool(name="kvb", bufs=2) as kvb, \
             tc.tile_pool(name="psA1", bufs=2, space="PSUM") as psA:
            xt_s = phA.tile([P, FT, TQ], bf16)
            xt_r = xT.rearrange("(t p) n -> p t n", p=P)
            for k in range(FT):
                nc.sync.dma_start(xt_s[:, k, :], xt_r[:, k, :])
            wk_r = wkTs.rearrange("(t p) m -> p t m", p=P)
            wv_r = wvT.rearrange("(t p) m -> p t m", p=P)
            # small consts after the critical-path xt (sync queue order)
            nc.sync.dma_start(bq_s[:], bqp)
            nc.sync.dma_start(bk_s[:], bkp)
            nc.sync.dma_start(bvB_s[:], bvB)
            nc.sync.dma_start(b1_s[:], b1p)
            nc.sync.dma_start(nri_s[:], nri)
            nc.sync.dma_start(nrm_s[:], nrm)
            for h in range(FT // 2):
                wk_c = wpool.tile([P, FT, 256], bf16, tag="w")
                nc.sync.dma_start(wk_c[:], wk_r[:, :, ts(h, 256)])
                wv_c = wpool.tile([P, FT, 256], bf16, tag="w")
                nc.sync.dma_start(wv_c[:], wv_r[:, :, ts(h, 256)])
                kts = []
                for mi in range(2):                 # K^T tiles t = 2h+mi
                    t = 2 * h + mi
                    ps = psA.tile([P, TQ], f32, tag="pa")
                    for k in range(FT):
                        nc.tensor.matmul(ps[:], wk_c[:, k, ts(mi, P)],
                                         xt_s[:, k, :],
                                         start=(k == 0), stop=(k == FT - 1))
                    ktl = kvb.tile([P, TQ], fp8, tag="kt")
                    nc.vector.tensor_scalar(
                        out=ktl[:], in0=ps[:], scalar1=bk_s[:, t:t + 1],
                        scalar2=None, op0=ALU.add)
                    kts.append(ktl)
                v2c = kvb.tile([P, 4, MT, HD], fp8, tag="v2")
                for m in range(MT):                 # V chunk (cols 256h..+256)
                    ps = psA.tile([P, 256], f32, tag="pa")
                    for k in range(FT):
                        nc.tensor.matmul(ps[:], xt_s[:, k, ts(m, P)],
                                         wv_c[:, k, :],
                                         start=(k == 0), stop=(k == FT - 1))
                    nc.vector.tensor_tensor(
                        out=v2c[:, :, m, :],
                        in0=ps.rearrange("p (a f) -> p a f", f=HD),
                        in1=bvB_s[:, ts(h, 256)]
                        .rearrange("p (a f) -> p a f", f=HD),
                        op=ALU.add)
                for mi in range(2):                 # bounce + AllGather
                    t = 2 * h + mi
                    nc.sync.dma_start(bounce[t][:, 0:TQ], kts[mi][:])
                    nc.sync.dma_start(
                        bounce[t][:, TQ:TQ + VB]
                        .rearrange("p (m f) -> p m f", f=HD),
                        v2c[:, 2 * mi, :, :])
                    nc.sync.dma_start(
                        bounce[t][:, TQ + VB:BW]
                        .rearrange("p (m f) -> p m f", f=HD),
                        v2c[:, 2 * mi + 1, :, :])
                    nc.gpsimd.collective_compute(
                        "AllGather", ALU.bypass, replica_groups=RG,
                        ins=[bounce[t][:]], outs=[agout[t][:]])

            # Q^T projection (overlaps the AllGather chain)
            wq_r = wqT.rearrange("(t p) m -> p t m", p=P)
            for half in range(2):
                wq_c = wpool.tile([P, FT, 512], bf16, tag="w")
                nc.sync.dma_start(wq_c[:], wq_r[:, :, ts(half, 512)])
                for mi in range(4):
                    t = 4 * half + mi
                    ps = psA.tile([P, TQ], f32, tag="pa")
                    for k in range(FT):
                        nc.tensor.matmul(ps[:], wq_c[:, k, ts(mi, P)],
                                         xt_s[:, k, :],
                                         start=(k == 0), stop=(k == FT - 1))
                    nc.vector.tensor_scalar(
                        out=qt_s[:, t, :], in0=ps[:],
                        scalar1=bq_s[:, t:t + 1], scalar2=None, op0=ALU.add)

        if DEBUG:
            nc.gpsimd.dma_start(dbg["dqt2"], qt_s[:])

        # ---------------- phase B: attention ----------------
        # vA = [V_h0 | 1] -> psC0 = [ctx0 ; d0 replicated]
        # vB = [1 | V_h1] -> psC1 = [d1 replicated ; ctx1]
        with tc.tile_pool(name="kvt", bufs=3) as kvt, \
             tc.tile_pool(name="esb", bufs=2) as esb, \
             tc.tile_pool(name="psS", bufs=2, space="PSUM") as psS, \
             tc.tile_pool(name="psC", bufs=2, space="PSUM") as psC, \
             tc.tile_pool(name="rec", bufs=2) as recp:
            for t in range(FT):          # head pair (2t, 2t+1)
                if t == 2:
                    # prefetch post-attention weights/consts now (DMA rings
                    # are past the AG-critical startup window)
                    nc.scalar.dma_start(xnb_s[:],
                                        xnb.rearrange("(m p) f -> p m f", p=P))
                    wo_r = woT.rearrange("(t p) m -> p t m", p=P)
                    w2_r = w2T.rearrange("(t p) m -> p t m", p=P)
                    for h in range(2):
                        nc.scalar.dma_start(wo_s[h][:],
                                            wo_r[:, :, ts(h, 512)])
                        nc.scalar.dma_start(w2_s[h][:],
                                            w2_r[:, :, ts(h, 512)])
                    nc.scalar.dma_start(g1B_s[:], g1B)
                    nc.scalar.dma_start(bb2B_s[:], bb2B)
                    nc.scalar.dma_start(g2B_s[:], g2B)
                    nc.scalar.dma_start(be2B_s[:], be2B)
                pc0 = psC.tile([P, TQ], f32, tag="c0")
                pc1 = psC.tile([P, TQ], f32, tag="c1")
                first = True
                for rb in range(GSIZE):
                    ktile = kvt.tile([P, TQ], fp8, tag="k")
                    nc.sync.dma_start(ktile[:], agout[t][rb, :, 0:TQ])
                    vA = vA_s[rb % 2]
                    vB = vB_s[rb % 2]
                    nc.sync.dma_start(
                        vA[:],
                        agout[t][rb, :, TQ:TQ + VB]
                        .rearrange("p (m f) -> p m f", f=HD))
                    nc.sync.dma_start(
                        vB[:],
                        agout[t][rb, :, TQ + VB:BW]
                        .rearrange("p (m f) -> p m f", f=HD))
                    for sj in range(MT):
                        last = (rb == GSIZE - 1 and sj == MT - 1)
                        ps = psS.tile([P, 2, TQ], f32, tag="s")
                        nc.tensor.matmul(ps[:, 0, :],
                                         ktile[0:HD, ts(sj, P)],
                                         qt_s[0:HD, t, :],
                                         start=True, stop=True)
                        nc.tensor.matmul(ps[:, 1, :],
                                         ktile[HD:P, ts(sj, P)],
                                         qt_s[HD:P, t, :],
                                         start=True, stop=True)
                        e = esb.tile([P, 2, NEX], fp8, tag="e", bufs=3)
                        nc.scalar.activation(e[:], ps[:, :, 0:NEX], AF.Exp)
                        # fast-exp (Schraudolph) for tail queries: one DVE op
                        # to int32; AV reads the high bf16 halves directly.
                        ei = esb.tile([P, 2, TQ - NEX], i32, tag="ei", bufs=3)
                        nc.vector.tensor_scalar(
                            out=ei[:], in0=ps[:, :, NEX:TQ],
                            scalar1=EXP_A, scalar2=EXP_B,
                            op0=ALU.mult, op1=ALU.add)
                        eb = ei.bitcast(bf16).rearrange(
                            "p h (n two) -> p h n two", two=2)
                        # col-tiled AV: V->ctx half and ones->denominator
                        # half run concurrently on distinct PE col groups;
                        # exact (n=384) and fast-exp tail (n=128) regions
                        # accumulate into disjoint PSUM column ranges.
                        nc.tensor.matmul(pc0[0:HD, 0:NEX], vA[:, sj, :],
                                         e[:, 0, :], start=first, stop=last)
                        nc.tensor.matmul(pc0[HD:P, 0:NEX], ones64[:],
                                         e[:, 0, :], start=first, stop=last)
                        nc.tensor.matmul(pc0[0:HD, NEX:TQ], vA[:, sj, :],
                                         eb[:, 0, :, 1], start=first,
                                         stop=last)
                        nc.tensor.matmul(pc0[HD:P, NEX:TQ], ones64[:],
                                         eb[:, 0, :, 1], start=first,
                                         stop=last)
                        nc.tensor.matmul(pc1[HD:P, 0:NEX], vB[:, sj, :],
                                         e[:, 1, :], start=first, stop=last)
                        nc.tensor.matmul(pc1[0:HD, 0:NEX], ones64[:],
                                         e[:, 1, :], start=first, stop=last)
                        nc.tensor.matmul(pc1[HD:P, NEX:TQ], vB[:, sj, :],
                                         eb[:, 1, :, 1], start=first,
                                         stop=last)
                        nc.tensor.matmul(pc1[0:HD, NEX:TQ], ones64[:],
                                         eb[:, 1, :, 1], start=first,
                                         stop=last)
                        first = False
                # normalize: d1 recip runs straight off PSUM at base 0; d0
                # is copied down (aligned), recip'd at base 0, both results
                # hopped (gpsimd DMA) to line up with the ctx partitions.
                dd = recp.tile([P, TQ], f32, tag="dd")
                nc.vector.tensor_copy(dd[HD:P, :], pc0[HD:P, :])
                nc.gpsimd.dma_start(dd[0:HD, :], dd[HD:P, :])
                rr = recp.tile([P, TQ], f32, tag="rr")
                nc.vector.reciprocal_approx_fast(rr[0:HD, :], pc1[0:HD, :])
                rec2 = recp.tile([P, TQ], f32, tag="r2")
                nc.vector.reciprocal_approx_fast(rec2[0:HD, :], dd[0:HD, :])
                nc.gpsimd.dma_start(rec2[HD:P, :], rr[0:HD, :])
                nc.vector.tensor_tensor(out=ctxT_s[0:HD, t, :],
                                        in0=pc0[0:HD, :], in1=rec2[0:HD, :],
                                        op=ALU.mult)
                nc.vector.tensor_tensor(out=ctxT_s[HD:P, t, :],
                                        in0=pc1[HD:P, :], in1=rec2[HD:P, :],
                                        op=ALU.mult)
                if DEBUG and t == 0:
                    pcs = recp.tile([P, 2, TQ], f32, tag="dbgpc")
                    nc.vector.tensor_copy(pcs[:, 0, :], pc0[:])
                    nc.vector.tensor_copy(pcs[:, 1, :], pc1[:])
                    nc.gpsimd.dma_start(dbg["dpc"], pcs[:])
                    nc.gpsimd.dma_start(dbg["drec"], rec2[:])
                    nc.gpsimd.dma_start(dbg["dag0"], agout[0][:])

        # ---------------- phases C-F ----------------
        with tc.tile_pool(name="lnp", bufs=2) as lnp, \
             tc.tile_pool(name="psB", bufs=2, space="PSUM") as psB, \
             tc.tile_pool(name="psT", bufs=2, space="PSUM") as psT:
            t1_s = acts.tile([P, MT, H], f32, tag="tres")

            def rstd_of(mv):
                ve = lnp.tile([P, 1], f32, tag="ve")
                nc.vector.tensor_scalar(out=ve[:], in0=mv[:, 1:2],
                                        scalar1=EPS, scalar2=None, op0=ALU.add)
                it = lnp.tile([P, 1], i32, tag="it")
                nc.vector.tensor_scalar(out=it[:], in0=ve.bitcast(i32),
                                        scalar1=nri_s[:, 0:1], scalar2=None,
                                        op0=ALU.logical_shift_right)
                nc.vector.tensor_scalar(out=it[:], in0=it[:],
                                        scalar1=nri_s[:, 1:2], scalar2=None,
                                        op0=ALU.bitwise_xor)
                nc.vector.tensor_tensor(out=it[:], in0=it[:], in1=nrm_s[:],
                                        op=ALU.add)
                rstd = it.bitcast(f32)
                nrt = lnp.tile([P, 1], f32, tag="nrt")
                for _ in range(2):
                    nc.vector.tensor_tensor(out=nrt[:], in0=rstd, in1=rstd,
                                            op=ALU.mult)
                    nc.vector.tensor_tensor(out=nrt[:], in0=nrt[:], in1=ve[:],
                                            op=ALU.mult)
                    nc.vector.tensor_scalar(out=nrt[:], in0=nrt[:],
                                            scalar1=-0.5, scalar2=1.5,
                                            op0=ALU.mult, op1=ALU.add)
                    nc.vector.tensor_tensor(out=rstd, in0=rstd, in1=nrt[:],
                                            op=ALU.mult)
                return rstd

            def ln_apply(src, dst, m):
                # dst = (src - mu) * rstd via the scalar engine:
                # out = src * rstd + (-mu * rstd)
                st = lnp.tile([P, 2, 6], f32, tag="st")
                nc.vector.bn_stats(out=st[:, 0, :], in_=src[:, m, 0:512])
                nc.vector.bn_stats(out=st[:, 1, :], in_=src[:, m, 512:H])
                mv = lnp.tile([P, 2], f32, tag="mv")
                nc.vector.bn_aggr(out=mv[:], in_=st[:])
                rstd = rstd_of(mv)
                nb = lnp.tile([P, 1], f32, tag="nb")
                nc.vector.tensor_scalar(out=nb[:], in0=mv[:, 0:1],
                                        scalar1=rstd, scalar2=-1.0,
                                        op0=ALU.mult, op1=ALU.mult)
                nc.scalar.activation(dst[:, m, :], src[:, m, :], AF.Identity,
                                     bias=nb[:], scale=rstd)

            def transposes_of(m):
                for ft in range(FT):
                    pt = psT.tile([P, P], bf16, tag="pt")
                    nc.tensor.transpose(pt[:], zn_s[:, m, ts(ft, P)], ident[:])
                    nc.vector.tensor_copy(ln1T_s[:, ft, ts(m, P)], pt[:])

            # C: Wo + residual, m-major; LN1 + transposes pipelined
            for m in range(MT):
                for half in range(2):
                    ps = psB.tile([P, 512], f32, tag="pb")
                    for k in range(FT):
                        nc.tensor.matmul(ps[:], ctxT_s[:, k, ts(m, P)],
                                         wo_s[half][:, k, :],
                                         start=(k == 0), stop=(k == FT - 1))
                    nc.vector.tensor_tensor(
                        out=t1_s[:, m, ts(half, 512)], in0=ps[:],
                        in1=xnb_s[:, m, ts(half, 512)], op=ALU.add)
                ln_apply(t1_s, zn_s, m)
                if m >= 1:
                    transposes_of(m - 1)
            transposes_of(MT - 1)
            if DEBUG:
                nc.gpsimd.dma_start(dbg["dt1"], t1_s[:])

            # residual side: ln1+b2 = zn*g1 + (be1+b2)   (gpsimd, off path)
            for m in range(MT):
                nc.gpsimd.tensor_tensor(out=ln1b2_s[:, m, :],
                                        in0=zn_s[:, m, :], in1=g1B_s[:],
                                        op=ALU.mult)
                nc.gpsimd.tensor_tensor(out=ln1b2_s[:, m, :],
                                        in0=ln1b2_s[:, m, :], in1=bb2B_s[:],
                                        op=ALU.add)

            # E: fc1 + relu (gamma folded into w1T host-side)
            w1_r = w1T.rearrange("(t p) m -> p t m", p=P)
            for half in range(2):
                w1_c = wpool.tile([P, FT, 512], bf16, tag="w")
                nc.sync.dma_start(w1_c[:], w1_r[:, :, ts(half, 512)])
                for mi in range(4):
                    mt_i = 4 * half + mi
                    ps = psB.tile([P, TQ], f32, tag="pb")
                    for k in range(FT):
                        nc.tensor.matmul(ps[:], w1_c[:, k, ts(mi, P)],
                                         ln1T_s[:, k, :],
                                         start=(k == 0), stop=(k == FT - 1))
                    nc.scalar.activation(hT_s[:, mt_i, :], ps[:], AF.Relu,
                                         bias=b1_s[:, mt_i:mt_i + 1])

            # F: fc2 + residual + LN2 + out, m-major pipelined
            t2_s = acts.tile([P, MT, H], f32, tag="tres")
            y_r = y.rearrange("(m p) f -> p m f", p=P)
            for m in range(MT):
                for half in range(2):
                    ps = psB.tile([P, 512], f32, tag="pb")
                    for k in range(FT):
                        nc.tensor.matmul(ps[:], hT_s[:, k, ts(m, P)],
                                         w2_s[half][:, k, :],
                                         start=(k == 0), stop=(k == FT - 1))
                    nc.vector.tensor_tensor(
                        out=t2_s[:, m, ts(half, 512)], in0=ps[:],
                        in1=ln1b2_s[:, m, ts(half, 512)], op=ALU.add)
                ln_apply(t2_s, t2_s, m)
                nc.vector.tensor_tensor(out=t2_s[:, m, :], in0=t2_s[:, m, :],
                                        in1=g2B_s[:], op=ALU.mult)
                nc.vector.tensor_tensor(out=t2_s[:, m, :], in0=t2_s[:, m, :],
                                        in1=be2B_s[:], op=ALU.add)
                nc.sync.dma_start(y_r[:, m, :], t2_s[:, m, :])
            if DEBUG:
                nc.gpsimd.dma_start(dbg["dqt"], qt_s[:])
                nc.gpsimd.dma_start(dbg["dctx"], ctxT_s[:])
                nc.gpsimd.dma_start(dbg["dzn"], zn_s[:])
                nc.gpsimd.dma_start(dbg["dht"], hT_s[:])

    nc.compile()
    return nc


_NC_CACHE = {}


def _get_nc():
    if 'nc' not in _NC_CACHE:
        _NC_CACHE['nc'] = build_kernel()
    return _NC_CACHE['nc']


def _bf(a):
    return np.ascontiguousarray(np.asarray(a, np.float32)).astype(
        ml_dtypes.bfloat16)


def make_in_maps(x, Wq, bq, Wk, bk, Wv, bv, Wo, bo, W1, b1, W2, b2,
                 g1, be1, g2, be2):
    def pt(b):  # [H] -> [P, FT] partition-tiled
        return np.ascontiguousarray(np.asarray(b, np.float32).reshape(FT, P).T)

    def bc(v):  # [H] -> [P, H] broadcast f32
        return np.ascontiguousarray(
            np.broadcast_to(np.asarray(v, np.float32), (P, H)))

    def bcb(v):  # [H] -> [P, H] broadcast bf16
        return np.ascontiguousarray(
            np.broadcast_to(np.asarray(v, np.float32).astype(
                ml_dtypes.bfloat16), (P, H)))

    scale = np.float32(1.0 / np.sqrt(HD))
    W1f = np.asarray(W1, np.float32)
    g1f = np.asarray(g1, np.float32)
    be1f = np.asarray(be1, np.float32)
    shared = {
        "wqT": _bf(np.asarray(Wq, np.float32).T),
        "wkTs": _bf(np.asarray(Wk, np.float32).T * scale),
        "wvT": _bf(np.asarray(Wv, np.float32).T),
        "woT": _bf(np.asarray(Wo, np.float32).T),
        "w1T": _bf((W1f * g1f[None, :]).T),
        "w2T": _bf(np.asarray(W2, np.float32).T),
        "bqp": pt(bq),
        "bkp": pt(np.asarray(bk, np.float32) * scale),
        "b1p": pt(np.asarray(b1, np.float32) + W1f @ be1f),
        "bvB": bc(bv),
        "g1B": bcb(g1),
        "bb2B": bcb(be1f + np.asarray(b2, np.float32)),
        "g2B": bc(g2), "be2B": bc(be2),
        "nri": np.tile(np.array([[1, -1]], np.int32), (P, 1)),
        "nrm": np.full((P, 1), 0x5f3759df + 1, np.int32),
    }
    in_maps = []
    for c in range(NCORES):
        b, sl = c // GSIZE, (c % GSIZE) * TQ
        xs = np.asarray(x[b, sl:sl + TQ, :], np.float32)
        m = dict(shared)
        m["xT"] = _bf(xs.T)
        m["xnb"] = np.ascontiguousarray(xs + np.asarray(bo, np.float32))
        in_maps.append(m)
    return in_maps


def kernel(x, Wq, bq, Wk, bk, Wv, bv, Wo, bo, W1, b1, W2, b2,
           g1, be1, g2, be2):
    x = np.asarray(x)
    nc = _get_nc()
    in_maps = make_in_maps(x, Wq, bq, Wk, bk, Wv, bv, Wo, bo,
                           W1, b1, W2, b2, g1, be1, g2, be2)
    res = run_bass_kernel_spmd(nc, in_maps, list(range(NCORES)))
    out = np.empty((B, S, H), np.float32)
    for c in range(NCORES):
        b, sl = c // GSIZE, (c % GSIZE) * TQ
        out[b, sl:sl + TQ, :] = res.results[c]["y"]
    return out
